# revision 9
# baseline (speedup 1.0000x reference)
"""Trainium2 Bass kernel: sliding-window GQA attention block.

Computation (matches the PyTorch/JAX reference):
    q,k,v = x @ {Wq,Wk,Wv}.T ; QK-RMSNorm ; RoPE ; GQA repeat(4x) ;
    softmax(q k^T / sqrt(D) + sliding-window bias(|i-j|<=512)) v ; @ Wo.T

Sharding (no collectives): 8 cores = 2 batches x 4 sequence chunks of 512
tokens.  Each core computes its 512 own tokens for ALL 16 heads, using a
512-token halo either side for K/V (halo K/V recomputed locally), then the
full o_proj rows for its tokens.  Outputs concatenate on host.

Layouts: projections contract over hidden, so both operands feed the PE
with hidden on partitions (host pre-transposes x and W).  Q/K are produced
directly in [head_dim, tokens] layout so attention scores^T and PV matmuls
need no on-device transposes; RMSNorm partition-dim reductions are done
with ones-vector matmuls; softmax normalization is applied after PV via a
PE-broadcast of the reciprocal denominators.  Matmul inputs use the fp32r
(reduced-mantissa fp32) PE mode: full-rate with moving dim 512.
"""

import numpy as np


def _ensure_path():
    try:
        import concourse  # noqa: F401
    except ImportError:
        import sys
        for p in ("/opt/trn_rl_repo", "/root/.axon_site/_ro/trn_rl_repo"):
            if p not in sys.path:
                sys.path.insert(0, p)


H, KV, D = 16, 4, 128
GQ = H // KV            # 4 query heads per kv head
WIN = 512
EPS = 1e-6
B, L, HID = 2, 2048, 2048
OWN = 512               # tokens owned per core
HALO = 1536             # key/value token window per core (own +- 512)
NKT = HALO // 128       # 12 key tiles of 128
NHK = HID // 128        # 16 contraction tiles over hidden
N_CORES = 8
FMIN = np.finfo(np.float32).min
# key-tile indices whose scores need the additive mask (band edges +
# sequence-validity); tiles 4..7 are fully in-window and valid for every core
BIAS_KT = (0, 1, 2, 3, 8, 9, 10, 11)

_CACHE = {}


def _build():
    _ensure_path()
    import concourse.mybir as mybir
    import concourse.tile as tile
    from concourse import bacc
    from contextlib import ExitStack

    F32 = mybir.dt.float32
    F32R = mybir.dt.float32r
    ACTF = mybir.ActivationFunctionType

    nc = bacc.Bacc("TRN2", target_bir_lowering=False, debug=False,
                   num_devices=N_CORES)

    xT = nc.dram_tensor("xT", [HID, HALO], F32R, kind="ExternalInput").ap()
    WqT = nc.dram_tensor("WqT", [HID, H * D], F32R, kind="ExternalInput").ap()
    WkT = nc.dram_tensor("WkT", [HID, KV * D], F32R, kind="ExternalInput").ap()
    WvT = nc.dram_tensor("WvT", [HID, KV * D], F32R, kind="ExternalInput").ap()
    WoT = nc.dram_tensor("WoT", [H * D, HID], F32R, kind="ExternalInput").ap()
    # RoPE tables, transposed to [D, tokens], norm-weights (and for q the
    # 1/sqrt(D) score scale) folded in; s-table has rotate_half sign/roll.
    cqT = nc.dram_tensor("cqT", [D, OWN], F32, kind="ExternalInput").ap()
    sqT = nc.dram_tensor("sqT", [D, OWN], F32, kind="ExternalInput").ap()
    ckT = nc.dram_tensor("ckT", [D, HALO], F32, kind="ExternalInput").ap()
    skT = nc.dram_tensor("skT", [D, HALO], F32, kind="ExternalInput").ap()
    bias8 = nc.dram_tensor("bias8", [len(BIAS_KT), 128, OWN], F32,
                           kind="ExternalInput").ap()
    out = nc.dram_tensor("out", [OWN, HID], F32, kind="ExternalOutput").ap()

    with tile.TileContext(nc) as tc, ExitStack() as top:
        # ---- persistent SBUF ----
        keep = top.enter_context(tc.tile_pool(name="keep", bufs=1))
        v_sb = keep.tile([128, NKT, KV * D], F32R)      # [tok128, ktile, vf]
        kT_sb = keep.tile([128, KV, HALO], F32R)        # [d, kv, tok]
        qT_sb = keep.tile([128, H, OWN], F32R)          # [d, h, tok]
        aoT_sb = keep.tile([128, H, OWN], F32R)         # [d, h, tok]
        ones32 = keep.tile([128, 1], F32)
        nc.vector.memset(ones32, 1.0)
        ones_sb = keep.tile([128, 1], F32R)
        nc.vector.tensor_copy(ones_sb, ones32)
        ones132 = keep.tile([1, 128], F32)
        nc.vector.memset(ones132, 1.0)
        ones1_sb = keep.tile([1, 128], F32R)
        nc.vector.tensor_copy(ones1_sb, ones132)

        # ================= V projection =================
        with ExitStack() as ph:
            sb = ph.enter_context(tc.tile_pool(name="vw", bufs=1))
            xs = ph.enter_context(tc.tile_pool(name="vx", bufs=4))
            ps = ph.enter_context(tc.tile_pool(name="vps", bufs=8,
                                               space="PSUM"))
            wv_sb = sb.tile([128, NHK, KV * D], F32R)
            nc.gpsimd.dma_start(out=wv_sb,
                                in_=WvT.rearrange("(k p) n -> p k n", p=128))
            for ch in range(3):
                pv = [ps.tile([128, KV * D], F32, tag="pv", name=f"pv{t}")
                      for t in range(4)]
                for k in range(NHK):
                    xt = xs.tile([128, 512], F32R, tag="xt")
                    nc.scalar.dma_start(
                        out=xt, in_=xT[k * 128:(k + 1) * 128,
                                       ch * 512:(ch + 1) * 512])
                    for tt in range(4):
                        nc.tensor.matmul(pv[tt], xt[:, tt * 128:(tt + 1) * 128],
                                         wv_sb[:, k, :],
                                         start=(k == 0), stop=(k == NHK - 1))
                for tt in range(4):
                    nc.scalar.copy(out=v_sb[:, ch * 4 + tt, :], in_=pv[tt])

        # ============ K / Q projection + RMSNorm + RoPE ============
        # head dims are host-interleaved [0,64,1,65,...]: rotate_half pairs
        # sit on adjacent partitions, so the swap is one DVE stream-shuffle
        SWAP_MASK = [p ^ 1 for p in range(32)]

        def norm_rope(p_feat, cT, sT, r_dst, n_tok, scratch, psn):
            """p_feat: psum [128 d, n_tok] raw head; writes r_dst (fp32r)."""
            raw = scratch.tile([128, n_tok], F32, tag="raw")
            nc.scalar.copy(out=raw, in_=p_feat)
            sq = scratch.tile([128, n_tok], F32R, tag="sq")
            nc.vector.tensor_mul(out=sq, in0=raw, in1=raw)
            pss = psn.tile([1, n_tok], F32, tag="ss")
            nc.tensor.matmul(pss, ones_sb, sq, start=True, stop=True)
            ms = scratch.tile([1, n_tok], F32, tag="ms")
            nc.vector.tensor_scalar(out=ms, in0=pss, scalar1=1.0 / D,
                                    scalar2=EPS, op0=mybir.AluOpType.mult,
                                    op1=mybir.AluOpType.add)
            nc.vector.reciprocal(ms, ms)
            rs = scratch.tile([1, n_tok], F32R, tag="rs")
            nc.scalar.activation(out=rs, in_=ms, func=ACTF.Sqrt)
            prb = psn.tile([128, n_tok], F32, tag="rb")
            nc.tensor.matmul(prb, ones1_sb, rs, start=True, stop=True)
            swp = scratch.tile([128, n_tok], F32, tag="swp")
            nc.vector.stream_shuffle(out=swp, in_=raw, mask=SWAP_MASK)
            t1 = scratch.tile([128, n_tok], F32, tag="t1")
            nc.vector.tensor_mul(out=t1, in0=raw, in1=cT)
            t2 = scratch.tile([128, n_tok], F32, tag="t2")
            nc.vector.tensor_mul(out=t2, in0=swp, in1=sT)
            nc.vector.tensor_add(out=t1, in0=t1, in1=t2)
            nc.vector.tensor_mul(out=r_dst, in0=t1, in1=prb)

        with ExitStack() as ph:
            sb = ph.enter_context(tc.tile_pool(name="kw", bufs=1))
            xs = ph.enter_context(tc.tile_pool(name="kx", bufs=4))
            psk = ph.enter_context(tc.tile_pool(name="kps", bufs=4,
                                                space="PSUM"))
            psn = ph.enter_context(tc.tile_pool(name="kpsn", bufs=2,
                                                space="PSUM"))
            scratch = ph.enter_context(tc.tile_pool(name="ksc", bufs=2))
            wk_sb = sb.tile([128, NHK, KV * D], F32R)
            nc.gpsimd.dma_start(out=wk_sb,
                                in_=WkT.rearrange("(k p) n -> p k n", p=128))
            ck_sb = sb.tile([128, HALO], F32)
            sk_sb = sb.tile([128, HALO], F32)
            nc.gpsimd.dma_start(out=ck_sb, in_=ckT)
            nc.gpsimd.dma_start(out=sk_sb, in_=skT)
            for ch in range(3):
                pk = [psk.tile([128, 512], F32, tag="pk", name=f"pk{t}")
                      for t in range(KV)]
                for k in range(NHK):
                    xt = xs.tile([128, 512], F32R, tag="xt")
                    nc.sync.dma_start(
                        out=xt, in_=xT[k * 128:(k + 1) * 128,
                                       ch * 512:(ch + 1) * 512])
                    for h in range(KV):
                        nc.tensor.matmul(pk[h],
                                         wk_sb[:, k, h * 128:(h + 1) * 128],
                                         xt, start=(k == 0),
                                         stop=(k == NHK - 1))
                sl = slice(ch * 512, (ch + 1) * 512)
                for h in range(KV):
                    norm_rope(pk[h], ck_sb[:, sl], sk_sb[:, sl],
                              kT_sb[:, h, sl], 512, scratch, psn)

        with ExitStack() as ph:
            sb = ph.enter_context(tc.tile_pool(name="qw", bufs=1))
            ws = ph.enter_context(tc.tile_pool(name="qwq", bufs=4))
            psq = ph.enter_context(tc.tile_pool(name="qps", bufs=4,
                                                space="PSUM"))
            psn = ph.enter_context(tc.tile_pool(name="qpsn", bufs=2,
                                                space="PSUM"))
            scratch = ph.enter_context(tc.tile_pool(name="qsc", bufs=2))
            xo_sb = sb.tile([128, NHK, OWN], F32R)
            nc.gpsimd.dma_start(
                out=xo_sb,
                in_=xT[:, 512:1024].rearrange("(k p) n -> p k n", p=128))
            cq_sb = sb.tile([128, OWN], F32)
            sq_sb = sb.tile([128, OWN], F32)
            nc.gpsimd.dma_start(out=cq_sb, in_=cqT)
            nc.gpsimd.dma_start(out=sq_sb, in_=sqT)
            for qf in range(4):
                pq = [psq.tile([128, OWN], F32, tag="pq", name=f"pq{t}")
                      for t in range(4)]
                for k in range(NHK):
                    wq = ws.tile([128, 512], F32R, tag="wq")
                    nc.gpsimd.dma_start(
                        out=wq, in_=WqT[k * 128:(k + 1) * 128,
                                        qf * 512:(qf + 1) * 512])
                    for j in range(4):
                        nc.tensor.matmul(pq[j],
                                         wq[:, j * 128:(j + 1) * 128],
                                         xo_sb[:, k, :], start=(k == 0),
                                         stop=(k == NHK - 1))
                for j in range(4):
                    norm_rope(pq[j], cq_sb, sq_sb, qT_sb[:, qf * 4 + j, :],
                              OWN, scratch, psn)

        # ================= attention =================
        with ExitStack() as ph:
            pss = ph.enter_context(tc.tile_pool(name="aps", bufs=2,
                                                space="PSUM"))
            pso = ph.enter_context(tc.tile_pool(name="apo", bufs=2,
                                                space="PSUM"))
            psd = ph.enter_context(tc.tile_pool(name="apd", bufs=2,
                                                space="PSUM"))
            psb = ph.enter_context(tc.tile_pool(name="apb", bufs=1,
                                                space="PSUM"))
            es = ph.enter_context(tc.tile_pool(name="aes", bufs=3))
            sc = ph.enter_context(tc.tile_pool(name="asc", bufs=2))
            bp = ph.enter_context(tc.tile_pool(name="abp", bufs=1))
            bias_sb = bp.tile([128, len(BIAS_KT), OWN], F32)
            nc.gpsimd.dma_start(out=bias_sb,
                                in_=bias8.rearrange("b p q -> p b q"))
            for h in range(H):
                kv = h // GQ
                po = pso.tile([128, OWN], F32, tag="po")
                pd = psd.tile([1, OWN], F32, tag="pd")
                for kt in range(NKT):
                    pscr = pss.tile([128, OWN], F32, tag="ps")
                    nc.tensor.matmul(pscr,
                                     kT_sb[:, kv, kt * 128:(kt + 1) * 128],
                                     qT_sb[:, h, :], start=True, stop=True)
                    if kt in BIAS_KT:
                        idx = BIAS_KT.index(kt)
                        nc.vector.tensor_add(out=pscr, in0=pscr,
                                             in1=bias_sb[:, idx, :])
                    e = es.tile([128, OWN], F32R, tag="e")
                    nc.scalar.activation(out=e, in_=pscr, func=ACTF.Exp)
                    nc.tensor.matmul(po, v_sb[:, kt, kv * 128:(kv + 1) * 128],
                                     e, start=(kt == 0), stop=(kt == NKT - 1))
                    nc.tensor.matmul(pd, ones_sb, e,
                                     start=(kt == 0), stop=(kt == NKT - 1))
                dr32 = sc.tile([1, OWN], F32, tag="dr32")
                nc.vector.reciprocal(dr32, pd)
                dr = sc.tile([1, OWN], F32R, tag="dr")
                nc.vector.tensor_copy(dr, dr32)
                pb = psb.tile([128, OWN], F32, tag="pb")
                nc.tensor.matmul(pb, ones1_sb, dr, start=True, stop=True)
                bf = sc.tile([128, OWN], F32, tag="bf")
                nc.scalar.copy(out=bf, in_=pb)
                nc.vector.tensor_mul(out=aoT_sb[:, h, :], in0=po, in1=bf)

        # ================= output projection =================
        with ExitStack() as ph:
            ws = ph.enter_context(tc.tile_pool(name="ow", bufs=4))
            psy = ph.enter_context(tc.tile_pool(name="ops", bufs=8,
                                                space="PSUM"))
            ys = ph.enter_context(tc.tile_pool(name="oy", bufs=4))
            for hc in range(4):
                py = [psy.tile([128, 512], F32, tag="py", name=f"py{t}")
                      for t in range(4)]
                for h in range(H):
                    wo = ws.tile([128, 512], F32R, tag="wo")
                    nc.gpsimd.dma_start(
                        out=wo, in_=WoT[h * 128:(h + 1) * 128,
                                        hc * 512:(hc + 1) * 512])
                    for tt in range(4):
                        nc.tensor.matmul(py[tt],
                                         aoT_sb[:, h, tt * 128:(tt + 1) * 128],
                                         wo, start=(h == 0), stop=(h == H - 1))
                for tt in range(4):
                    y = ys.tile([128, 512], F32, tag="y")
                    nc.scalar.copy(out=y, in_=py[tt])
                    nc.sync.dma_start(
                        out=out[tt * 128:(tt + 1) * 128,
                                hc * 512:(hc + 1) * 512], in_=y)

    nc.compile()
    return nc


def _host_prep(x, cos, sin, Wq, Wk, Wv, Wo, q_norm_w, k_norm_w):
    """Build the 8 per-core input dicts."""
    scale = 1.0 / np.sqrt(D)
    # interleave head dims [0,64,1,65,...]: rotate_half partners end up on
    # adjacent partitions so the kernel swaps them with one stream-shuffle
    perm = np.empty(D, np.int64)
    perm[0::2] = np.arange(64)
    perm[1::2] = 64 + np.arange(64)

    def rope_tables(cos_r, sin_r, w, extra):
        # fold norm weight (and any extra scale); sign/roll for rotate_half
        c = (cos_r * w[None, :] * extra).astype(np.float32)
        w_rot = np.roll(w, -64)
        s = (sin_r * w_rot[None, :] * extra).astype(np.float32)
        s[:, :64] *= -1.0
        return (np.ascontiguousarray(c.T[perm]),
                np.ascontiguousarray(s.T[perm]))

    idx_q = (np.arange(H)[:, None] * D + perm[None, :]).ravel()
    idx_k = (np.arange(KV)[:, None] * D + perm[None, :]).ravel()
    WqT = np.ascontiguousarray(Wq.T[:, idx_q])
    WkT = np.ascontiguousarray(Wk.T[:, idx_k])
    WvT = np.ascontiguousarray(Wv.T)
    WoT = np.ascontiguousarray(Wo.T)

    in_maps = []
    for c in range(N_CORES):
        b, ch = divmod(c, 4)
        start = ch * OWN
        lo, hi = start - WIN, start + OWN + WIN
        vlo, vhi = max(lo, 0), min(hi, L)
        xh = np.zeros((HALO, HID), np.float32)
        xh[vlo - lo:vhi - lo] = x[b, vlo:vhi]
        ch_cos = np.zeros((HALO, D), np.float32)
        ch_sin = np.zeros((HALO, D), np.float32)
        ch_cos[vlo - lo:vhi - lo] = cos[vlo:vhi]
        ch_sin[vlo - lo:vhi - lo] = sin[vlo:vhi]
        ckT, skT = rope_tables(ch_cos, ch_sin, k_norm_w, 1.0)
        cqT, sqT = rope_tables(cos[start:start + OWN], sin[start:start + OWN],
                               q_norm_w, scale)
        # additive mask for the 8 edge key-tiles: [8, 128 k, OWN q]
        q_glob = start + np.arange(OWN)[None, :]
        bias = np.empty((len(BIAS_KT), 128, OWN), np.float32)
        for i, kt in enumerate(BIAS_KT):
            k_glob = (lo + kt * 128 + np.arange(128))[:, None]
            ok = (np.abs(k_glob - q_glob) <= WIN) & (k_glob >= 0) & (k_glob < L)
            bias[i] = np.where(ok, 0.0, FMIN)
        in_maps.append({
            "xT": np.ascontiguousarray(xh.T),
            "WqT": WqT, "WkT": WkT, "WvT": WvT, "WoT": WoT,
            "cqT": cqT, "sqT": sqT, "ckT": ckT, "skT": skT,
            "bias8": bias,
        })
    return in_maps


def kernel(**inputs):
    _ensure_path()
    from concourse import bass_utils

    if "nc" not in _CACHE:
        _CACHE["nc"] = _build()
    nc = _CACHE["nc"]

    in_maps = _host_prep(
        np.asarray(inputs["x"]), np.asarray(inputs["cos"]),
        np.asarray(inputs["sin"]), np.asarray(inputs["Wq"]),
        np.asarray(inputs["Wk"]), np.asarray(inputs["Wv"]),
        np.asarray(inputs["Wo"]), np.asarray(inputs["q_norm_w"]),
        np.asarray(inputs["k_norm_w"]))

    res = bass_utils.run_bass_kernel_spmd(nc, in_maps,
                                          core_ids=list(range(N_CORES)))
    out = np.empty((B, L, HID), np.float32)
    for c in range(N_CORES):
        b, ch = divmod(c, 4)
        out[b, ch * OWN:(ch + 1) * OWN] = res.results[c]["out"]
    return out


# revision 21
# speedup vs baseline: 1.7668x; 1.7668x over previous
"""Trainium2 Bass kernel: sliding-window GQA attention block.

Computation (matches the PyTorch/JAX reference):
    q,k,v = x @ {Wq,Wk,Wv}.T ; QK-RMSNorm ; RoPE ; GQA repeat(4x) ;
    softmax(q k^T / sqrt(D) + sliding-window bias(|i-j|<=512)) v ; @ Wo.T

Sharding (no collectives): 8 cores = 2 batches x 4 sequence chunks of 512
tokens.  Each core computes its 512 own tokens for ALL 16 heads, using a
512-token halo either side for K/V (halo K/V recomputed locally), then the
full o_proj rows for its tokens.  Outputs concatenate on host.

Layouts: projections contract over hidden, so both operands feed the PE
with hidden on partitions (host pre-transposes x and W).  Q/K are produced
directly in [head_dim, tokens] layout so attention scores^T and PV matmuls
need no on-device transposes; RMSNorm partition-dim reductions are done
with ones-vector matmuls; softmax normalization is applied after PV via a
PE-broadcast of the reciprocal denominators.  Matmul inputs use the fp32r
(reduced-mantissa fp32) PE mode: full-rate with moving dim >= 256.

Scheduling: resident tensors live in two alternating SBUF zones
(A: Wv -> x_own -> attn_out, B: Wk -> bias) so a phase's weights prefetch
on the GPSIMD/SWDGE path while the previous phase computes, instead of
stalling on the zone's previous readers.  Head dims are host-interleaved
[0,64,1,65,...] so RoPE's rotate_half is a single DVE stream-shuffle.
"""

import numpy as np


def _ensure_path():
    try:
        import concourse  # noqa: F401
    except ImportError:
        import sys
        for p in ("/opt/trn_rl_repo", "/root/.axon_site/_ro/trn_rl_repo"):
            if p not in sys.path:
                sys.path.insert(0, p)


H, KV, D = 16, 4, 128
GQ = H // KV            # 4 query heads per kv head
WIN = 512
EPS = 1e-6
B, L, HID = 2, 2048, 2048
OWN = 512               # tokens owned per core
HALO = 1536             # key/value token window per core (own +- 512)
NKT = HALO // 128       # 12 key tiles of 128
NHK = HID // 128        # 16 contraction tiles over hidden
N_CORES = 8
FMIN = np.finfo(np.float32).min
# key-tile indices whose scores need the additive mask (band edges +
# sequence-validity); tiles 4..7 are fully in-window and valid for every core
BIAS_KT = (0, 1, 2, 3, 8, 9, 10, 11)
# (key-tile, q_start, q_width): edge tiles only overlap the window for half
# the queries, so they run at half width.  kt=2 goes first: its start=True
# initialises every psum column.
KT_PLAN = [(2, 0, 512), (3, 0, 512), (4, 0, 512), (5, 0, 512),
           (6, 0, 512), (7, 0, 512), (8, 0, 512), (9, 0, 512),
           (10, 256, 256), (11, 256, 256), (0, 0, 256), (1, 0, 256)]

_CACHE = {}


def _build():
    _ensure_path()
    import concourse.mybir as mybir
    import concourse.tile as tile
    from concourse import bacc
    from contextlib import ExitStack

    F32 = mybir.dt.float32
    F32R = mybir.dt.float32r
    ACTF = mybir.ActivationFunctionType

    nc = bacc.Bacc("TRN2", target_bir_lowering=False, debug=False,
                   num_devices=N_CORES)

    xT = nc.dram_tensor("xT", [HID, HALO], F32R, kind="ExternalInput").ap()
    WqT = nc.dram_tensor("WqT", [HID, H * D], F32R, kind="ExternalInput").ap()
    WkT = nc.dram_tensor("WkT", [HID, KV * D], F32R, kind="ExternalInput").ap()
    WvT = nc.dram_tensor("WvT", [HID, KV * D], F32R, kind="ExternalInput").ap()
    WoT = nc.dram_tensor("WoT", [H * D, HID], F32R, kind="ExternalInput").ap()
    # RoPE tables, transposed to [D, tokens], norm-weights (and for q the
    # 1/sqrt(D) score scale) folded in; s-table has rotate_half sign/roll.
    cqT = nc.dram_tensor("cqT", [D, OWN], F32, kind="ExternalInput").ap()
    sqT = nc.dram_tensor("sqT", [D, OWN], F32, kind="ExternalInput").ap()
    ckT = nc.dram_tensor("ckT", [D, HALO], F32, kind="ExternalInput").ap()
    skT = nc.dram_tensor("skT", [D, HALO], F32, kind="ExternalInput").ap()
    bias8 = nc.dram_tensor("bias8", [len(BIAS_KT), 128, OWN], F32,
                           kind="ExternalInput").ap()
    out = nc.dram_tensor("out", [OWN, HID], F32, kind="ExternalOutput").ap()

    SWAP_MASK = [p ^ 1 for p in range(32)]

    with tile.TileContext(nc) as tc, ExitStack() as top:
        # ---- persistent SBUF ----
        keep = top.enter_context(tc.tile_pool(name="keep", bufs=1))
        v_sb = keep.tile([128, NKT, KV * D], F32R)      # [tok128, ktile, vf]
        kT_sb = keep.tile([128, KV, HALO], F32R)        # [d, kv, tok]
        qT_sb = keep.tile([128, H, OWN], F32R)          # [d, h, tok]
        ones32 = keep.tile([128, 1], F32)
        nc.vector.memset(ones32, 1.0)
        ones_sb = keep.tile([128, 1], F32R)
        nc.vector.tensor_copy(ones_sb, ones32)
        ones132 = keep.tile([1, 128], F32)
        nc.vector.memset(ones132, 1.0)
        ones1_sb = keep.tile([1, 128], F32R)
        nc.vector.tensor_copy(ones1_sb, ones132)

        # alternating resident zones: a phase's tensors prefetch while the
        # *other* zone's previous-phase readers drain
        zoneA = top.enter_context(tc.tile_pool(name="zoneA", bufs=1))
        zoneB = top.enter_context(tc.tile_pool(name="zoneB", bufs=1))
        xs = top.enter_context(tc.tile_pool(name="xs", bufs=3))
        ws = top.enter_context(tc.tile_pool(name="ws", bufs=5))

        def load_sliced(dst, src, n):
            for k in range(n):
                nc.gpsimd.dma_start(out=dst[:, k, :],
                                    in_=src[k * 128:(k + 1) * 128, :])

        # ================= V projection =================
        wv_sb = zoneA.tile([128, NHK, KV * D], F32R, tag="big", name="wv_sb")
        load_sliced(wv_sb, WvT, NHK)
        with ExitStack() as ph:
            ps = ph.enter_context(tc.tile_pool(name="vps", bufs=8,
                                               space="PSUM"))
            for ch in range(3):
                pv = [ps.tile([128, KV * D], F32, tag="pv", name=f"pv{t}")
                      for t in range(4)]
                for k in range(NHK):
                    xt = xs.tile([128, 512], F32R, tag="xt")
                    nc.sync.dma_start(
                        out=xt, in_=xT[k * 128:(k + 1) * 128,
                                       ch * 512:(ch + 1) * 512])
                    for tt in range(4):
                        nc.tensor.matmul(pv[tt], xt[:, tt * 128:(tt + 1) * 128],
                                         wv_sb[:, k, :],
                                         start=(k == 0), stop=(k == NHK - 1))
                for tt in range(4):
                    nc.scalar.copy(out=v_sb[:, ch * 4 + tt, :], in_=pv[tt])

        # ============ K / Q projection + RMSNorm + RoPE ============
        def norm_rope(p_feat, cT, sT, r_dst, n_tok, psn, scratch):
            """p_feat: psum [128 d, n_tok] raw head; writes r_dst (fp32r)."""
            sq = scratch.tile([128, n_tok], F32R, tag="sq")
            nc.scalar.activation(out=sq, in_=p_feat, func=ACTF.Square)
            raw = scratch.tile([128, n_tok], F32, tag="raw")
            nc.scalar.copy(out=raw, in_=p_feat)
            pss = psn.tile([1, n_tok], F32, tag="ss")
            nc.tensor.matmul(pss, ones_sb, sq, start=True, stop=True)
            ms = scratch.tile([1, n_tok], F32, tag="ms")
            nc.vector.tensor_scalar(out=ms, in0=pss, scalar1=1.0 / D,
                                    scalar2=EPS, op0=mybir.AluOpType.mult,
                                    op1=mybir.AluOpType.add)
            nc.vector.reciprocal(ms, ms)
            rs = scratch.tile([1, n_tok], F32R, tag="rs")
            nc.scalar.activation(out=rs, in_=ms, func=ACTF.Sqrt)
            prb = psn.tile([128, n_tok], F32, tag="rb")
            nc.tensor.matmul(prb, ones1_sb, rs, start=True, stop=True)
            swp = scratch.tile([128, n_tok], F32, tag="swp")
            nc.vector.stream_shuffle(out=swp, in_=raw, mask=SWAP_MASK)
            t1 = scratch.tile([128, n_tok], F32, tag="t1")
            nc.gpsimd.tensor_mul(out=t1, in0=raw, in1=cT)
            t2 = scratch.tile([128, n_tok], F32, tag="t2")
            nc.gpsimd.tensor_mul(out=t2, in0=swp, in1=sT)
            nc.gpsimd.tensor_add(out=t1, in0=t1, in1=t2)
            nc.vector.tensor_mul(out=r_dst, in0=t1, in1=prb)

        wk_sb = zoneB.tile([128, NHK, KV * D], F32R, tag="big", name="wk_sb")
        load_sliced(wk_sb, WkT, NHK)
        ck_sb = zoneB.tile([128, HALO], F32, tag="tc", name="ck_sb")
        sk_sb = zoneB.tile([128, HALO], F32, tag="ts", name="sk_sb")
        nc.gpsimd.dma_start(out=ck_sb, in_=ckT)
        nc.gpsimd.dma_start(out=sk_sb, in_=skT)

        with ExitStack() as ph:
            psp = ph.enter_context(tc.tile_pool(name="psp", bufs=6,
                                                space="PSUM"))
            psn = ph.enter_context(tc.tile_pool(name="psn", bufs=1,
                                                space="PSUM"))
            scratch = ph.enter_context(tc.tile_pool(name="scratch", bufs=2))
            for ch in range(3):
                pk = [psp.tile([128, 512], F32, tag="p", name=f"pk{t}")
                      for t in range(KV)]
                for k in range(NHK):
                    xt = xs.tile([128, 512], F32R, tag="xt")
                    nc.sync.dma_start(
                        out=xt, in_=xT[k * 128:(k + 1) * 128,
                                       ch * 512:(ch + 1) * 512])
                    for h in range(KV):
                        nc.tensor.matmul(pk[h],
                                         wk_sb[:, k, h * 128:(h + 1) * 128],
                                         xt, start=(k == 0),
                                         stop=(k == NHK - 1))
                sl = slice(ch * 512, (ch + 1) * 512)
                for h in range(KV):
                    norm_rope(pk[h], ck_sb[:, sl], sk_sb[:, sl],
                              kT_sb[:, h, sl], 512, psn, scratch)

            # ---- Q: x_own reuses zone A (Wv readers are done) ----
            xo_sb = zoneA.tile([128, NHK, OWN], F32R, tag="big", name="xo_sb")
            load_sliced(xo_sb, xT[:, 512:1024], NHK)
            cq_sb = zoneA.tile([128, OWN], F32, tag="tc", name="cq_sb")
            sq_sb = zoneA.tile([128, OWN], F32, tag="ts", name="sq_sb")
            nc.gpsimd.dma_start(out=cq_sb, in_=cqT)
            nc.gpsimd.dma_start(out=sq_sb, in_=sqT)
            for qf in range(4):
                pq = [psp.tile([128, OWN], F32, tag="p", name=f"pq{t}")
                      for t in range(4)]
                for k in range(NHK):
                    wq = ws.tile([128, 512], F32R, tag="w")
                    nc.sync.dma_start(
                        out=wq, in_=WqT[k * 128:(k + 1) * 128,
                                        qf * 512:(qf + 1) * 512])
                    for j in range(4):
                        nc.tensor.matmul(pq[j],
                                         wq[:, j * 128:(j + 1) * 128],
                                         xo_sb[:, k, :], start=(k == 0),
                                         stop=(k == NHK - 1))
                for j in range(4):
                    norm_rope(pq[j], cq_sb, sq_sb, qT_sb[:, qf * 4 + j, :],
                              OWN, psn, scratch)

        # ================= attention =================
        bias_sb = zoneB.tile([128, len(BIAS_KT), OWN], F32, tag="big",
                             name="bias_sb")
        for i in range(len(BIAS_KT)):
            nc.gpsimd.dma_start(out=bias_sb[:, i, :], in_=bias8[i])
        aoT_sb = zoneA.tile([128, H, OWN], F32R, tag="big", name="aoT_sb")
        with ExitStack() as ph:
            pss = ph.enter_context(tc.tile_pool(name="aps", bufs=3,
                                                space="PSUM"))
            pso = ph.enter_context(tc.tile_pool(name="apo", bufs=2,
                                                space="PSUM"))
            psd = ph.enter_context(tc.tile_pool(name="apd", bufs=2,
                                                space="PSUM"))
            psb = ph.enter_context(tc.tile_pool(name="apb", bufs=1,
                                                space="PSUM"))
            es = ph.enter_context(tc.tile_pool(name="aes", bufs=6))
            sc = ph.enter_context(tc.tile_pool(name="asc", bufs=3))
            for h in range(H):
                kv = h // GQ
                po = pso.tile([128, OWN], F32, tag="po")
                pd = psd.tile([1, OWN], F32, tag="pd")
                for n_kt, (kt, q0, qw) in enumerate(KT_PLAN):
                    qsl = slice(q0, q0 + qw)
                    pscr = pss.tile([128, OWN], F32, tag="ps")
                    nc.tensor.matmul(pscr[:, :qw],
                                     kT_sb[:, kv, kt * 128:(kt + 1) * 128],
                                     qT_sb[:, h, qsl], start=True, stop=True)
                    if kt in BIAS_KT:
                        idx = BIAS_KT.index(kt)
                        nc.vector.tensor_add(out=pscr[:, :qw],
                                             in0=pscr[:, :qw],
                                             in1=bias_sb[:, idx, qsl])
                    e = es.tile([128, OWN], F32R, tag="e")
                    nc.scalar.activation(out=e[:, :qw], in_=pscr[:, :qw],
                                         func=ACTF.Exp)
                    nc.tensor.matmul(po[:, qsl],
                                     v_sb[:, kt, kv * 128:(kv + 1) * 128],
                                     e[:, :qw], start=(n_kt == 0),
                                     stop=(n_kt == NKT - 1),
                                     skip_group_check=True)
                    nc.tensor.matmul(pd[:, qsl], ones_sb, e[:, :qw],
                                     start=(n_kt == 0), stop=(n_kt == NKT - 1),
                                     skip_group_check=True)
                dr32 = sc.tile([1, OWN], F32, tag="dr32")
                nc.vector.reciprocal(dr32, pd)
                dr = sc.tile([1, OWN], F32R, tag="dr")
                nc.vector.tensor_copy(dr, dr32)
                pb = psb.tile([128, OWN], F32, tag="pb")
                nc.tensor.matmul(pb, ones1_sb, dr, start=True, stop=True)
                bf = sc.tile([128, OWN], F32, tag="bf")
                nc.vector.tensor_copy(bf, pb)
                nc.vector.tensor_mul(out=aoT_sb[:, h, :], in0=po, in1=bf)

        # ================= output projection =================
        with ExitStack() as ph:
            psy = ph.enter_context(tc.tile_pool(name="ops", bufs=8,
                                                space="PSUM"))
            ys = ph.enter_context(tc.tile_pool(name="oy", bufs=4))
            for hc in range(4):
                py = [psy.tile([128, 512], F32, tag="py", name=f"py{t}")
                      for t in range(4)]
                for h in range(H):
                    wo = ws.tile([128, 512], F32R, tag="w")
                    eng = nc.sync if h % 2 == 0 else nc.scalar
                    eng.dma_start(
                        out=wo, in_=WoT[h * 128:(h + 1) * 128,
                                        hc * 512:(hc + 1) * 512])
                    for tt in range(4):
                        nc.tensor.matmul(py[tt],
                                         aoT_sb[:, h, tt * 128:(tt + 1) * 128],
                                         wo, start=(h == 0), stop=(h == H - 1))
                for tt in range(4):
                    y = ys.tile([128, 512], F32, tag="y")
                    nc.scalar.copy(out=y, in_=py[tt])
                    nc.sync.dma_start(
                        out=out[tt * 128:(tt + 1) * 128,
                                hc * 512:(hc + 1) * 512], in_=y)

    nc.compile()
    return nc


def _host_prep(x, cos, sin, Wq, Wk, Wv, Wo, q_norm_w, k_norm_w):
    """Build the 8 per-core input dicts."""
    scale = 1.0 / np.sqrt(D)
    # interleave head dims [0,64,1,65,...]: rotate_half partners end up on
    # adjacent partitions so the kernel swaps them with one stream-shuffle
    perm = np.empty(D, np.int64)
    perm[0::2] = np.arange(64)
    perm[1::2] = 64 + np.arange(64)

    def rope_tables(cos_r, sin_r, w, extra):
        # fold norm weight (and any extra scale); sign/roll for rotate_half
        c = (cos_r * w[None, :] * extra).astype(np.float32)
        w_rot = np.roll(w, -64)
        s = (sin_r * w_rot[None, :] * extra).astype(np.float32)
        s[:, :64] *= -1.0
        return (np.ascontiguousarray(c.T[perm]),
                np.ascontiguousarray(s.T[perm]))

    idx_q = (np.arange(H)[:, None] * D + perm[None, :]).ravel()
    idx_k = (np.arange(KV)[:, None] * D + perm[None, :]).ravel()
    WqT = np.ascontiguousarray(Wq.T[:, idx_q])
    WkT = np.ascontiguousarray(Wk.T[:, idx_k])
    WvT = np.ascontiguousarray(Wv.T)
    WoT = np.ascontiguousarray(Wo.T)

    in_maps = []
    for c in range(N_CORES):
        b, ch = divmod(c, 4)
        start = ch * OWN
        lo, hi = start - WIN, start + OWN + WIN
        vlo, vhi = max(lo, 0), min(hi, L)
        xh = np.zeros((HALO, HID), np.float32)
        xh[vlo - lo:vhi - lo] = x[b, vlo:vhi]
        ch_cos = np.zeros((HALO, D), np.float32)
        ch_sin = np.zeros((HALO, D), np.float32)
        ch_cos[vlo - lo:vhi - lo] = cos[vlo:vhi]
        ch_sin[vlo - lo:vhi - lo] = sin[vlo:vhi]
        ckT, skT = rope_tables(ch_cos, ch_sin, k_norm_w, 1.0)
        cqT, sqT = rope_tables(cos[start:start + OWN], sin[start:start + OWN],
                               q_norm_w, scale)
        # additive mask for the 8 edge key-tiles: [8, 128 k, OWN q]
        q_glob = start + np.arange(OWN)[None, :]
        bias = np.empty((len(BIAS_KT), 128, OWN), np.float32)
        for i, kt in enumerate(BIAS_KT):
            k_glob = (lo + kt * 128 + np.arange(128))[:, None]
            ok = (np.abs(k_glob - q_glob) <= WIN) & (k_glob >= 0) & (k_glob < L)
            bias[i] = np.where(ok, 0.0, FMIN)
        in_maps.append({
            "xT": np.ascontiguousarray(xh.T),
            "WqT": WqT, "WkT": WkT, "WvT": WvT, "WoT": WoT,
            "cqT": cqT, "sqT": sqT, "ckT": ckT, "skT": skT,
            "bias8": bias,
        })
    return in_maps


def kernel(**inputs):
    _ensure_path()
    from concourse import bass_utils

    if "nc" not in _CACHE:
        _CACHE["nc"] = _build()
    nc = _CACHE["nc"]

    in_maps = _host_prep(
        np.asarray(inputs["x"]), np.asarray(inputs["cos"]),
        np.asarray(inputs["sin"]), np.asarray(inputs["Wq"]),
        np.asarray(inputs["Wk"]), np.asarray(inputs["Wv"]),
        np.asarray(inputs["Wo"]), np.asarray(inputs["q_norm_w"]),
        np.asarray(inputs["k_norm_w"]))

    res = bass_utils.run_bass_kernel_spmd(nc, in_maps,
                                          core_ids=list(range(N_CORES)))
    out = np.empty((B, L, HID), np.float32)
    for c in range(N_CORES):
        b, ch = divmod(c, 4)
        out[b, ch * OWN:(ch + 1) * OWN] = res.results[c]["out"]
    return out
